# revision 17
# baseline (speedup 1.0000x reference)
"""Trainium2 Bass kernel for FlashMultiHeadAttention (B=2, L=2048, D=1024, H=16, Dh=64).

Sharding: 8 cores = 2 (batch) x 4 (head groups of 4 heads).
Each core computes, for its batch element b and head group hg:
  - Q^T, K^T (transposed projections, [256, L]) and U, V (natural, [L, 256])
    via augmented-contraction matmuls that fold in biases and the action gate.
  - S^T = K^T.T-style scores transposed ([k, q]) so the learned time-delta bias
    is a per-partition ACT bias fused into the exp instruction.
  - PV with an extra ones column on gated V producing softmax denominators.
  - Row-sliced output projection -> partial outT [1024, 2048] (fp32).
Host sums the 4 head-group partials per batch and adds bo.
"""

import sys

if "/opt/trn_rl_repo" not in sys.path:
    sys.path.insert(0, "/opt/trn_rl_repo")

import numpy as np
import ml_dtypes

BF16 = ml_dtypes.bfloat16

B = 2
L = 2048
D = 1024
H = 16
DH = 64
NG = 256          # head dims per group (4 heads)
DPAD = 1152       # padded contraction rows (9 * 128)
NCORES = 8
SCALE = DH ** -0.5


def build_bass(with_mask: bool):
    """Build the single-core SPMD Bass program (same program on all 8 cores)."""
    import concourse.mybir as mybir
    from concourse import bacc
    from concourse.tile import TileContext

    f32 = mybir.dt.float32
    bf16 = mybir.dt.bfloat16
    EXP = mybir.ActivationFunctionType.Exp

    nc = bacc.Bacc(None, target_bir_lowering=False)

    xq = nc.dram_tensor("xq", [DPAD, L], bf16, kind="ExternalInput")
    xk = nc.dram_tensor("xk", [DPAD, L], bf16, kind="ExternalInput")
    xv = nc.dram_tensor("xv", [DPAD, L], bf16, kind="ExternalInput")
    wq = nc.dram_tensor("wq", [DPAD, NG], bf16, kind="ExternalInput")
    wqr = nc.dram_tensor("wqr", [DPAD, NG], bf16, kind="ExternalInput")
    wu = nc.dram_tensor("wu", [DPAD, NG], bf16, kind="ExternalInput")
    wk = nc.dram_tensor("wk", [DPAD, NG], bf16, kind="ExternalInput")
    wkr = nc.dram_tensor("wkr", [DPAD, NG], bf16, kind="ExternalInput")
    wv = nc.dram_tensor("wv", [DPAD, NG], bf16, kind="ExternalInput")
    wo = nc.dram_tensor("wo", [NG, D], bf16, kind="ExternalInput")
    cb = nc.dram_tensor("cb", [128, 64], f32, kind="ExternalInput")
    cs = nc.dram_tensor("cs", [128, L], f32, kind="ExternalInput")
    sn = nc.dram_tensor("sn", [128, L], f32, kind="ExternalInput")
    mk = None
    if with_mask:
        mk = nc.dram_tensor("mk", [L, L], f32, kind="ExternalInput")
    outT = nc.dram_tensor("outT", [D, L], f32, kind="ExternalOutput")

    dma = nc.default_dma_engine

    with TileContext(nc) as tc:
        with tc.tile_pool(name="persist", bufs=1) as persist:
            qT0 = persist.tile([128, L], bf16, name="qT0")
            qT1 = persist.tile([128, L], bf16, name="qT1")
            kT0 = persist.tile([128, L], bf16, name="kT0")
            kT1 = persist.tile([128, L], bf16, name="kT1")
            qT = [qT0, qT1]
            kT = [kT0, kT1]
            # gated V, 16 k-tiles x (4 heads x 65) cols; col 64 of each head
            # block is the all-ones denominator column.
            vg = persist.tile([128, 16 * 260], bf16, name="vg")
            oT0 = persist.tile([128, L], bf16, name="oT0")
            oT1 = persist.tile([128, L], bf16, name="oT1")
            oT = [oT0, oT1]
            sig0 = persist.tile([128, 1024], f32, name="sig0")
            sig1 = persist.tile([128, 1024], f32, name="sig1")
            sig2 = persist.tile([128, 1024], f32, name="sig2")
            sig3 = persist.tile([128, 1024], f32, name="sig3")
            sig = [sig0, sig1, sig2, sig3]
            cbS = persist.tile([128, 64], f32, name="cbS")
            dma.dma_start(out=cbS, in_=cb[:, :])
            csS = persist.tile([128, L], f32, name="csS")
            dma.dma_start(out=csS, in_=cs[:, :])
            snS = persist.tile([128, L], f32, name="snS")
            dma.dma_start(out=snS, in_=sn[:, :])
            woS0 = persist.tile([128, D], bf16, name="woS0")
            woS1 = persist.tile([128, D], bf16, name="woS1")
            woS = [woS0, woS1]
            for n2 in range(2):
                dma.dma_start(out=woS[n2], in_=wo[n2 * 128:(n2 + 1) * 128, :])

            vg4 = vg.rearrange("p (t h e) -> p t h e", h=4, e=65)
            nc.vector.memset(vg4[:, :, :, 64:65], 1.0)

            # ---- Phase QU: Q^T projection + U projection + sigmoid(U) ----
            with tc.tile_pool(name="wqu", bufs=1) as wpool, \
                 tc.tile_pool(name="xqu", bufs=1) as xpool, \
                 tc.tile_pool(name="psA", bufs=1, space="PSUM") as psA, \
                 tc.tile_pool(name="evA", bufs=2) as evpool:
                wqS = wpool.tile([128, 9 * NG], bf16, name="wqS")
                wqrS = wpool.tile([128, 9 * NG], bf16, name="wqrS")
                wuS = wpool.tile([128, 9 * NG], bf16, name="wuS")
                dma.dma_start(out=wqS.rearrange("p (c n) -> p c n", n=NG),
                              in_=wq.rearrange("(c p) n -> p c n", p=128))
                dma.dma_start(out=wqrS.rearrange("p (c n) -> p c n", n=NG),
                              in_=wqr.rearrange("(c p) n -> p c n", p=128))
                dma.dma_start(out=wuS.rearrange("p (c n) -> p c n", n=NG),
                              in_=wu.rearrange("(c p) n -> p c n", p=128))
                wqS3 = wqS.rearrange("p (c n) -> p c n", n=NG)
                wqrS3 = wqrS.rearrange("p (c n) -> p c n", n=NG)
                wuS3 = wuS.rearrange("p (c n) -> p c n", n=NG)
                xqS = xpool.tile([128, 9 * L], bf16, name="xqS")
                xqS3 = xqS.rearrange("p (c q) -> p c q", q=L)
                for d in range(9):
                    dma.dma_start(out=xqS3[:, d, :],
                                  in_=xq[d * 128:(d + 1) * 128, :])
                for qc in range(4):
                    s = slice(qc * 512, (qc + 1) * 512)
                    qps = [psA.tile([128, 512], f32, tag="q", bufs=4, name=f"qps{n}")
                           for n in range(2)]
                    qrps = [psA.tile([128, 512], f32, tag="q", bufs=4, name=f"qrps{n}")
                            for n in range(2)]
                    ups = [psA.tile([128, 256], f32, tag="u", bufs=4, name=f"ups{i}")
                           for i in range(4)]
                    for d in range(9):
                        xt = xqS3[:, d, s]
                        for n in range(2):
                            nc.tensor.matmul(qps[n], lhsT=wqS3[:, d, n * 128:(n + 1) * 128],
                                             rhs=xt, start=(d == 0), stop=(d == 8))
                            nc.tensor.matmul(qrps[n], lhsT=wqrS3[:, d, n * 128:(n + 1) * 128],
                                             rhs=xt, start=(d == 0), stop=(d == 8))
                        for i in range(4):
                            nc.tensor.matmul(ups[i],
                                             lhsT=xt[:, i * 128:(i + 1) * 128],
                                             rhs=wuS3[:, d, :],
                                             start=(d == 0), stop=(d == 8))
                    for n in range(2):
                        tc_ = evpool.tile([128, 512], f32, tag="tc", name="tc_")
                        tr_ = evpool.tile([128, 512], f32, tag="tr", name="tr_")
                        nc.vector.tensor_mul(tc_, qps[n], csS[:, s])
                        nc.vector.tensor_mul(tr_, qrps[n], snS[:, s])
                        nc.vector.tensor_add(qT[n][:, s], tc_, tr_)
                    eu = evpool.tile([128, 1024], f32, tag="eu", name="eu")
                    for i in range(4):
                        nc.scalar.activation(out=eu[:, i * 256:(i + 1) * 256],
                                             in_=ups[i], func=EXP, scale=-1.0)
                    nc.vector.tensor_scalar_add(eu, eu, 1.0)
                    nc.vector.reciprocal(out=sig[qc], in_=eu)

            # ---- Phase KV: K^T projection + V projection + gating ----
            with tc.tile_pool(name="wkv", bufs=1) as wpool, \
                 tc.tile_pool(name="xkv", bufs=1) as xpool, \
                 tc.tile_pool(name="psK", bufs=1, space="PSUM") as psK:
                wkS = wpool.tile([128, 9 * NG], bf16, name="wkS")
                wkrS = wpool.tile([128, 9 * NG], bf16, name="wkrS")
                wvS = wpool.tile([128, 9 * NG], bf16, name="wvS")
                dma.dma_start(out=wkS.rearrange("p (c n) -> p c n", n=NG),
                              in_=wk.rearrange("(c p) n -> p c n", p=128))
                dma.dma_start(out=wkrS.rearrange("p (c n) -> p c n", n=NG),
                              in_=wkr.rearrange("(c p) n -> p c n", p=128))
                dma.dma_start(out=wvS.rearrange("p (c n) -> p c n", n=NG),
                              in_=wv.rearrange("(c p) n -> p c n", p=128))
                wkS3 = wkS.rearrange("p (c n) -> p c n", n=NG)
                wkrS3 = wkrS.rearrange("p (c n) -> p c n", n=NG)
                wvS3 = wvS.rearrange("p (c n) -> p c n", n=NG)
                xkS = xpool.tile([128, 9 * L], bf16, name="xkS")
                xkS3 = xkS.rearrange("p (c q) -> p c q", q=L)
                xvS = xpool.tile([128, 9 * L], bf16, name="xvS")
                xvS3 = xvS.rearrange("p (c q) -> p c q", q=L)
                for d in range(9):
                    dma.dma_start(out=xkS3[:, d, :],
                                  in_=xk[d * 128:(d + 1) * 128, :])
                    dma.dma_start(out=xvS3[:, d, :],
                                  in_=xv[d * 128:(d + 1) * 128, :])
                for qc in range(4):
                    s = slice(qc * 512, (qc + 1) * 512)
                    kps = [psK.tile([128, 512], f32, tag="k", bufs=4, name=f"kps{n}")
                           for n in range(2)]
                    krps = [psK.tile([128, 512], f32, tag="k", bufs=4, name=f"krps{n}")
                            for n in range(2)]
                    vps = [psK.tile([128, 256], f32, tag="v", bufs=4, name=f"vps{i}")
                           for i in range(4)]
                    for d in range(9):
                        xtk = xkS3[:, d, s]
                        xtv = xvS3[:, d, s]
                        for n in range(2):
                            nc.tensor.matmul(kps[n], lhsT=wkS3[:, d, n * 128:(n + 1) * 128],
                                             rhs=xtk, start=(d == 0), stop=(d == 8))
                            nc.tensor.matmul(krps[n], lhsT=wkrS3[:, d, n * 128:(n + 1) * 128],
                                             rhs=xtk, start=(d == 0), stop=(d == 8))
                        for i in range(4):
                            nc.tensor.matmul(vps[i],
                                             lhsT=xtv[:, i * 128:(i + 1) * 128],
                                             rhs=wvS3[:, d, :],
                                             start=(d == 0), stop=(d == 8))
                    for n in range(2):
                        tc_ = wpool.tile([128, 512], f32, tag="ktc", bufs=2, name="tc_")
                        tr_ = wpool.tile([128, 512], f32, tag="ktr", bufs=2, name="tr_")
                        nc.vector.tensor_mul(tc_, kps[n], csS[:, s])
                        nc.vector.tensor_mul(tr_, krps[n], snS[:, s])
                        nc.vector.tensor_add(kT[n][:, s], tc_, tr_)
                    for i in range(4):
                        kt_g = qc * 4 + i
                        vsrc = vps[i].rearrange("p (h e) -> p h e", e=64)
                        ssrc = sig[qc][:, i * 256:(i + 1) * 256].rearrange(
                            "p (h e) -> p h e", e=64)
                        nc.vector.tensor_mul(vg4[:, kt_g, :, 0:64], vsrc, ssrc)

            # ---- Phase attention ----
            with tc.tile_pool(name="psB", bufs=1, space="PSUM") as psB, \
                 tc.tile_pool(name="ptp", bufs=3) as ptpool, \
                 tc.tile_pool(name="nrm", bufs=2) as nrmpool, \
                 tc.tile_pool(name="drm", bufs=2, space="DRAM") as drmpool, \
                 tc.tile_pool(name="mkp", bufs=4) as mkpool:
                for h in range(4):
                    n = h // 2
                    r0 = (h % 2) * 64
                    pvt = [psB.tile([65, 512], f32, tag="pv", bufs=4, name=f"pvt{qc}")
                           for qc in range(4)]
                    for kt in range(16):
                        for hq in range(2):
                            st = psB.tile([128, 1024], f32, tag="st", bufs=2, name="st")
                            for s2 in range(2):
                                q0 = hq * 1024 + s2 * 512
                                nc.tensor.matmul(
                                    st[:, s2 * 512:(s2 + 1) * 512],
                                    lhsT=kT[n][r0:r0 + 64, kt * 128:(kt + 1) * 128],
                                    rhs=qT[n][r0:r0 + 64, q0:q0 + 512],
                                    start=True, stop=True)
                            if with_mask:
                                mt = mkpool.tile([128, 1024], f32, tag="mt", name="mt")
                                dma.dma_start(
                                    out=mt,
                                    in_=mk[kt * 128:(kt + 1) * 128,
                                           hq * 1024:(hq + 1) * 1024])
                                nc.vector.tensor_add(st, st, mt)
                            pt = ptpool.tile([128, 1024], bf16, tag="pt", name="pt")
                            nc.scalar.activation(out=pt, in_=st, func=EXP,
                                                 scale=SCALE,
                                                 bias=cbS[:, kt * 4 + h:kt * 4 + h + 1])
                            for s2 in range(2):
                                qc = hq * 2 + s2
                                nc.tensor.matmul(
                                    pvt[qc],
                                    lhsT=vg[:, kt * 260 + h * 65:kt * 260 + h * 65 + 65],
                                    rhs=pt[:, s2 * 512:(s2 + 1) * 512],
                                    start=(kt == 0), stop=(kt == 15))
                    rv = nrmpool.tile([1, L], f32, tag="rv", name="rv")
                    for qc in range(4):
                        nc.vector.reciprocal(out=rv[0:1, qc * 512:(qc + 1) * 512],
                                             in_=pvt[qc][64:65, :])
                    drv = drmpool.tile([1, L], f32, tag="drv", name="drv")
                    dma.dma_start(out=drv, in_=rv)
                    ib = nrmpool.tile([64, L], f32, tag="ib", name="ib")
                    dma.dma_start(out=ib, in_=drv[0, :].partition_broadcast(64))
                    for qc in range(4):
                        nc.vector.tensor_mul(
                            oT[n][r0:r0 + 64, qc * 512:(qc + 1) * 512],
                            pvt[qc][0:64, :], ib[:, qc * 512:(qc + 1) * 512])

            # ---- Phase out-projection ----
            with tc.tile_pool(name="psC", bufs=1, space="PSUM") as psC, \
                 tc.tile_pool(name="otp", bufs=4) as otpool:
                for mt_i in range(8):
                    ot = otpool.tile([128, L], f32, tag="ot", bufs=2, name="ot")
                    for qc in range(4):
                        op = psC.tile([128, 512], f32, tag="op", bufs=4, name="op")
                        for n2 in range(2):
                            nc.tensor.matmul(
                                op,
                                lhsT=woS[n2][:, mt_i * 128:(mt_i + 1) * 128],
                                rhs=oT[n2][:, qc * 512:(qc + 1) * 512],
                                start=(n2 == 0), stop=(n2 == 1))
                        nc.vector.tensor_copy(out=ot[:, qc * 512:(qc + 1) * 512],
                                              in_=op)
                    dma.dma_start(out=outT[mt_i * 128:(mt_i + 1) * 128, :], in_=ot)

    nc.finalize()
    return nc


def prep_inputs(query, key, value, attn_mask, action_ids, time_deltas,
                Wq, bq, Wk, bk, Wv, bv, Wu, bu, Wo, bo,
                action_emb, Wap, bap, td_emb, td_gate):
    """Host-side sharding: build the 8 per-core input maps."""
    query = np.asarray(query, np.float32)
    key = np.asarray(key, np.float32)
    value = np.asarray(value, np.float32)
    attn_mask = np.asarray(attn_mask)
    action_ids = np.asarray(action_ids)
    time_deltas = np.asarray(time_deltas)

    sig_gate = 1.0 / (1.0 + np.exp(-np.float64(td_gate)))
    with_mask = not bool(attn_mask.all())

    # Per-batch transposed, augmented, bf16 input matrices.
    xq_b, xk_b, xv_b, cb_b, mk_b = [], [], [], [], []
    for b in range(B):
        ae = np.asarray(action_emb, np.float32)[action_ids[b]]      # [L, 16]
        xq = np.zeros((DPAD, L), BF16)
        xq[:D] = query[b].T.astype(BF16)
        xq[D:D + 16] = ae.T.astype(BF16)
        xq[D + 16] = BF16(1.0)
        xq_b.append(xq)
        xk = np.zeros((DPAD, L), BF16)
        xk[:D] = key[b].T.astype(BF16)
        xk[D] = BF16(1.0)
        xk_b.append(xk)
        xv = np.zeros((DPAD, L), BF16)
        xv[:D] = value[b].T.astype(BF16)
        xv[D] = BF16(1.0)
        xv_b.append(xv)
        tdc = np.clip(time_deltas[b].astype(np.int64), 0, td_emb.shape[0] - 1)
        cb_b.append((sig_gate * np.asarray(td_emb, np.float32)[tdc]).astype(np.float32))  # [L, H]
        if with_mask:
            m = np.where(attn_mask[b], np.float32(0.0), np.float32(-1e9))
            mk_b.append(np.ascontiguousarray(m.T))                  # [k, q]

    # Augmented weights (shared rows; per-core column/row slices).
    wq_a = np.zeros((DPAD, D), np.float32)
    wq_a[:D] = Wq
    wq_a[D + 16] = bq
    wu_a = np.zeros((DPAD, D), np.float32)
    wu_a[:D] = Wu
    wu_a[D:D + 16] = Wap
    wu_a[D + 16] = np.asarray(bu) + np.asarray(bap)
    wk_a = np.zeros((DPAD, D), np.float32)
    wk_a[:D] = Wk
    wk_a[D] = bk
    wv_a = np.zeros((DPAD, D), np.float32)
    wv_a[:D] = Wv
    wv_a[D] = bv

    # RoPE: rot(v)[d] = -v[d+32] (d<32) else v[d-32];  rot(v) = R @ v.
    # Fold R into a second projection weight per head block: W_rot = W @ R.T.
    R = np.zeros((DH, DH), np.float32)
    R[np.arange(32), np.arange(32) + 32] = -1.0
    R[np.arange(32) + 32, np.arange(32)] = 1.0
    def rot_fold(w):
        w3 = w.reshape(DPAD, H, DH)
        return np.einsum("dhe,fe->dhf", w3, R).reshape(DPAD, D)
    wqr_a = rot_fold(wq_a)
    wkr_a = rot_fold(wk_a)

    # cos/sin tables in [dh, pos] orientation, duplicated for the 2-head
    # partition packing (rows 0-63 and 64-127 identical).
    inv_freq = 1.0 / (10000.0 ** (np.arange(0, DH, 2, dtype=np.float64) / DH))
    pos = np.arange(L, dtype=np.float64)
    freqs = pos[None, :] * inv_freq[:, None]            # [32, L]
    cos_t = np.repeat(np.cos(freqs), 2, axis=0)[:DH]    # [64, L]
    sin_t = np.repeat(np.sin(freqs), 2, axis=0)[:DH]
    cs_t = np.ascontiguousarray(np.concatenate([cos_t, cos_t], 0), np.float32)
    sn_t = np.ascontiguousarray(np.concatenate([sin_t, sin_t], 0), np.float32)

    in_maps = []
    for c in range(NCORES):
        b, hg = c // 4, c % 4
        csl = slice(hg * NG, (hg + 1) * NG)
        cbc = cb_b[b][:, hg * 4:(hg + 1) * 4]                       # [L, 4]
        cbc = cbc.reshape(16, 128, 4).transpose(1, 0, 2).reshape(128, 64)
        m = {
            "xq": xq_b[b], "xk": xk_b[b], "xv": xv_b[b],
            "wq": wq_a[:, csl].astype(BF16), "wu": wu_a[:, csl].astype(BF16),
            "wk": wk_a[:, csl].astype(BF16), "wv": wv_a[:, csl].astype(BF16),
            "wqr": wqr_a[:, csl].astype(BF16), "wkr": wkr_a[:, csl].astype(BF16),
            "wo": np.asarray(Wo, np.float32)[csl, :].astype(BF16),
            "cb": np.ascontiguousarray(cbc, np.float32),
            "cs": cs_t, "sn": sn_t,
        }
        if with_mask:
            m["mk"] = mk_b[b]
        in_maps.append(m)
    return in_maps, with_mask


def gather_output(results, bo):
    """Sum head-group partials per batch, transpose, add bo."""
    out = np.empty((B, L, D), np.float32)
    for b in range(B):
        acc = results[b * 4]["outT"].astype(np.float32).copy()
        for g in range(1, 4):
            acc += results[b * 4 + g]["outT"]
        out[b] = acc.T + np.asarray(bo, np.float32)
    return out


def kernel(**inputs):
    from concourse.bass_utils import run_bass_kernel_spmd

    in_maps, with_mask = prep_inputs(**inputs)
    nc = build_bass(with_mask)
    res = run_bass_kernel_spmd(nc, in_maps, core_ids=list(range(NCORES)))
    return gather_output(res.results, inputs["bo"])


# revision 19
# speedup vs baseline: 1.1877x; 1.1877x over previous
"""Trainium2 Bass kernel for FlashMultiHeadAttention (B=2, L=2048, D=1024, H=16, Dh=64).

Sharding: 8 cores = 2 (batch) x 4 (head groups of 4 heads).
Per core (batch b, head group hg, 4 heads):
  - Q^T, K^T projections ([256, L], head dim on partitions) with RoPE applied
    during PSUM evacuation via partition-shifted DVE multiplies against
    cos / signed-sin tables; U, V projected in natural [L, 256] layout with
    biases and the action gate folded in via augmented contraction rows.
  - Scores computed transposed (S^T[k, q]) so the learned time-delta bias is a
    per-partition ACT bias fused into the exp instruction (scale+bias+exp+cast
    in one op). P^T feeds PV directly - no PE transposes anywhere.
  - PV carries an extra all-ones column producing softmax denominators;
    normalization uses a batched reciprocal + DRAM-broadcast of 1/r.
  - Row-sliced output projection -> partial outT [1024, 2048] fp32.
Host sums the 4 head-group partials per batch and adds bo.
"""

import sys

if "/opt/trn_rl_repo" not in sys.path:
    sys.path.insert(0, "/opt/trn_rl_repo")

import numpy as np
import ml_dtypes

BF16 = ml_dtypes.bfloat16

B = 2
L = 2048
D = 1024
H = 16
DH = 64
NG = 256          # head dims per group (4 heads)
DPAD = 1152       # padded contraction rows (9 * 128)
NCORES = 8
SCALE = DH ** -0.5


def build_bass(with_mask: bool):
    """Build the single-core SPMD Bass program (same program on all 8 cores)."""
    import concourse.mybir as mybir
    from concourse import bacc
    from concourse.tile import TileContext

    f32 = mybir.dt.float32
    bf16 = mybir.dt.bfloat16
    EXP = mybir.ActivationFunctionType.Exp

    nc = bacc.Bacc(None, target_bir_lowering=False)

    xq = nc.dram_tensor("xq", [DPAD, L], bf16, kind="ExternalInput")
    xk = nc.dram_tensor("xk", [DPAD, L], bf16, kind="ExternalInput")
    xv = nc.dram_tensor("xv", [DPAD, L], bf16, kind="ExternalInput")
    wq = nc.dram_tensor("wq", [DPAD, NG], bf16, kind="ExternalInput")
    wu = nc.dram_tensor("wu", [DPAD, NG], bf16, kind="ExternalInput")
    wk = nc.dram_tensor("wk", [DPAD, NG], bf16, kind="ExternalInput")
    wv = nc.dram_tensor("wv", [DPAD, NG], bf16, kind="ExternalInput")
    wo = nc.dram_tensor("wo", [NG, D], bf16, kind="ExternalInput")
    cb = nc.dram_tensor("cb", [128, 64], f32, kind="ExternalInput")
    cs = nc.dram_tensor("cs", [128, L], f32, kind="ExternalInput")
    sn = nc.dram_tensor("sn", [128, L], f32, kind="ExternalInput")
    mk = None
    if with_mask:
        mk = nc.dram_tensor("mk", [L, L], f32, kind="ExternalInput")
    outT = nc.dram_tensor("outT", [D, L], f32, kind="ExternalOutput")

    dma = nc.default_dma_engine

    def rope_evac(pps, dest, csS, snS, s, pool):
        """dest[:, s] (bf16) = pps*cos + rotate_half(pps)*signed_sin."""
        tc_ = pool.tile([128, 512], f32, tag="tc", name="tc_")
        tr_ = pool.tile([128, 512], f32, tag="tr", name="tr_")
        nc.vector.tensor_mul(tc_, pps, csS[:, s])
        for blk in (0, 64):
            nc.vector.tensor_mul(tr_[blk:blk + 32],
                                 pps[blk + 32:blk + 64, :], snS[blk:blk + 32, s])
            nc.vector.tensor_mul(tr_[blk + 32:blk + 64],
                                 pps[blk:blk + 32, :], snS[blk + 32:blk + 64, s])
        nc.vector.tensor_add(dest[:, s], tc_, tr_)

    with TileContext(nc) as tc:
        with tc.tile_pool(name="persist", bufs=1) as persist, \
             tc.tile_pool(name="xqp", bufs=1) as xqpool, \
             tc.tile_pool(name="xkp", bufs=1) as xkpool:
            qT = [persist.tile([128, L], bf16, name=f"qT{n}") for n in range(2)]
            kT = [persist.tile([128, L], bf16, name=f"kT{n}") for n in range(2)]
            # gated V, 16 k-tiles x (4 heads x 65); col 64 of each head block
            # is the all-ones denominator column.
            vg = persist.tile([128, 16 * 260], bf16, name="vg")
            oTu = [persist.tile([128, L], bf16, name=f"oTu{n}") for n in range(2)]
            oT = [persist.tile([128, L], bf16, name=f"oT{n}") for n in range(2)]
            sig = [persist.tile([128, 1024], f32, name=f"sig{qc}") for qc in range(4)]
            cbS = persist.tile([128, 64], f32, name="cbS")
            dma.dma_start(out=cbS, in_=cb[:, :])
            csS = persist.tile([128, L], f32, name="csS")
            dma.dma_start(out=csS, in_=cs[:, :])
            snS = persist.tile([128, L], f32, name="snS")
            dma.dma_start(out=snS, in_=sn[:, :])
            woS = [persist.tile([128, D], bf16, name=f"woS{n2}") for n2 in range(2)]
            for n2 in range(2):
                dma.dma_start(out=woS[n2], in_=wo[n2 * 128:(n2 + 1) * 128, :])

            vg4 = vg.rearrange("p (t h e) -> p t h e", h=4, e=65)
            nc.vector.memset(vg4[:, :, :, 64:65], 1.0)

            # X^T inputs for Q/U (xq) and K (xk) resident up-front.
            xqS = xqpool.tile([128, 9 * L], bf16, name="xqS")
            xqS3 = xqS.rearrange("p (c q) -> p c q", q=L)
            xkS = xkpool.tile([128, 9 * L], bf16, name="xkS")
            xkS3 = xkS.rearrange("p (c q) -> p c q", q=L)
            for d in range(9):
                dma.dma_start(out=xqS3[:, d, :], in_=xq[d * 128:(d + 1) * 128, :])
                dma.dma_start(out=xkS3[:, d, :], in_=xk[d * 128:(d + 1) * 128, :])

            # ---- Phase QU: Q^T projection (+RoPE) + U projection + sigmoid ----
            with tc.tile_pool(name="wqu", bufs=1) as wpool, \
                 tc.tile_pool(name="psA", bufs=1, space="PSUM") as psA, \
                 tc.tile_pool(name="evA", bufs=2) as evpool:
                wqS = wpool.tile([128, 9 * NG], bf16, name="wqS")
                wuS = wpool.tile([128, 9 * NG], bf16, name="wuS")
                dma.dma_start(out=wqS.rearrange("p (c n) -> p c n", n=NG),
                              in_=wq.rearrange("(c p) n -> p c n", p=128))
                dma.dma_start(out=wuS.rearrange("p (c n) -> p c n", n=NG),
                              in_=wu.rearrange("(c p) n -> p c n", p=128))
                wqS3 = wqS.rearrange("p (c n) -> p c n", n=NG)
                wuS3 = wuS.rearrange("p (c n) -> p c n", n=NG)
                for qc in range(4):
                    s = slice(qc * 512, (qc + 1) * 512)
                    qps = [psA.tile([128, 512], f32, tag="q", bufs=4, name=f"qps{n}")
                           for n in range(2)]
                    ups = [psA.tile([128, 256], f32, tag="u", bufs=4, name=f"ups{i}")
                           for i in range(4)]
                    for d in range(9):
                        xt = xqS3[:, d, s]
                        for n in range(2):
                            nc.tensor.matmul(qps[n], lhsT=wqS3[:, d, n * 128:(n + 1) * 128],
                                             rhs=xt, start=(d == 0), stop=(d == 8))
                        for i in range(4):
                            nc.tensor.matmul(ups[i],
                                             lhsT=xt[:, i * 128:(i + 1) * 128],
                                             rhs=wuS3[:, d, :],
                                             start=(d == 0), stop=(d == 8))
                    for n in range(2):
                        rope_evac(qps[n], qT[n], csS, snS, s, evpool)
                    eu = evpool.tile([128, 1024], f32, tag="eu", name="eu")
                    for i in range(4):
                        nc.scalar.activation(out=eu[:, i * 256:(i + 1) * 256],
                                             in_=ups[i], func=EXP, scale=-1.0)
                    nc.vector.tensor_scalar_add(eu, eu, 1.0)
                    nc.vector.reciprocal(out=sig[qc], in_=eu)

            # ---- Phase KV: K^T projection (+RoPE) + V projection + gating ----
            with tc.tile_pool(name="wkv", bufs=1) as wpool, \
                 tc.tile_pool(name="xvp", bufs=1) as xvpool, \
                 tc.tile_pool(name="psK", bufs=1, space="PSUM") as psK, \
                 tc.tile_pool(name="evK", bufs=2) as evpool:
                wkS = wpool.tile([128, 9 * NG], bf16, name="wkS")
                wvS = wpool.tile([128, 9 * NG], bf16, name="wvS")
                dma.dma_start(out=wkS.rearrange("p (c n) -> p c n", n=NG),
                              in_=wk.rearrange("(c p) n -> p c n", p=128))
                dma.dma_start(out=wvS.rearrange("p (c n) -> p c n", n=NG),
                              in_=wv.rearrange("(c p) n -> p c n", p=128))
                wkS3 = wkS.rearrange("p (c n) -> p c n", n=NG)
                wvS3 = wvS.rearrange("p (c n) -> p c n", n=NG)
                xvS = xvpool.tile([128, 9 * L], bf16, name="xvS")
                xvS3 = xvS.rearrange("p (c q) -> p c q", q=L)
                for d in range(9):
                    dma.dma_start(out=xvS3[:, d, :], in_=xv[d * 128:(d + 1) * 128, :])
                for qc in range(4):
                    s = slice(qc * 512, (qc + 1) * 512)
                    kps = [psK.tile([128, 512], f32, tag="k", bufs=4, name=f"kps{n}")
                           for n in range(2)]
                    vps = [psK.tile([128, 256], f32, tag="v", bufs=4, name=f"vps{i}")
                           for i in range(4)]
                    for d in range(9):
                        xtk = xkS3[:, d, s]
                        xtv = xvS3[:, d, s]
                        for n in range(2):
                            nc.tensor.matmul(kps[n], lhsT=wkS3[:, d, n * 128:(n + 1) * 128],
                                             rhs=xtk, start=(d == 0), stop=(d == 8))
                        for i in range(4):
                            nc.tensor.matmul(vps[i],
                                             lhsT=xtv[:, i * 128:(i + 1) * 128],
                                             rhs=wvS3[:, d, :],
                                             start=(d == 0), stop=(d == 8))
                    for n in range(2):
                        rope_evac(kps[n], kT[n], csS, snS, s, evpool)
                    for i in range(4):
                        kt_g = qc * 4 + i
                        vsrc = vps[i].rearrange("p (h e) -> p h e", e=64)
                        ssrc = sig[qc][:, i * 256:(i + 1) * 256].rearrange(
                            "p (h e) -> p h e", e=64)
                        nc.vector.tensor_mul(vg4[:, kt_g, :, 0:64], vsrc, ssrc)

            # ---- Phase attention ----
            with tc.tile_pool(name="psB", bufs=1, space="PSUM") as psB, \
                 tc.tile_pool(name="ptp", bufs=3) as ptpool, \
                 tc.tile_pool(name="nrm", bufs=2) as nrmpool, \
                 tc.tile_pool(name="drm", bufs=2, space="DRAM") as drmpool, \
                 tc.tile_pool(name="mkp", bufs=4) as mkpool:
                for h in range(4):
                    n = h // 2
                    r0 = (h % 2) * 64
                    pvt = [psB.tile([65, 512], f32, tag="pv", bufs=4, name=f"pvt{qc}")
                           for qc in range(4)]
                    for kt in range(16):
                        for hq in range(2):
                            st = psB.tile([128, 1024], f32, tag="st", bufs=2, name="st")
                            for s2 in range(2):
                                q0 = hq * 1024 + s2 * 512
                                nc.tensor.matmul(
                                    st[:, s2 * 512:(s2 + 1) * 512],
                                    lhsT=kT[n][r0:r0 + 64, kt * 128:(kt + 1) * 128],
                                    rhs=qT[n][r0:r0 + 64, q0:q0 + 512],
                                    start=True, stop=True)
                            if with_mask:
                                mt = mkpool.tile([128, 1024], f32, tag="mt", name="mt")
                                dma.dma_start(
                                    out=mt,
                                    in_=mk[kt * 128:(kt + 1) * 128,
                                           hq * 1024:(hq + 1) * 1024])
                                nc.vector.tensor_add(st, st, mt)
                            pt = ptpool.tile([128, 1024], bf16, tag="pt", name="pt")
                            nc.scalar.activation(out=pt, in_=st, func=EXP,
                                                 scale=SCALE,
                                                 bias=cbS[:, kt * 4 + h:kt * 4 + h + 1])
                            for s2 in range(2):
                                qc = hq * 2 + s2
                                nc.tensor.matmul(
                                    pvt[qc],
                                    lhsT=vg[:, kt * 260 + h * 65:kt * 260 + h * 65 + 65],
                                    rhs=pt[:, s2 * 512:(s2 + 1) * 512],
                                    start=(kt == 0), stop=(kt == 15))
                    # evacuate numerators (unnormalized) + denominators, free psum.
                    # Denominator rows land at 32-aligned partitions {0,32,64,96}.
                    rg = nrmpool.tile([128, 512], f32, tag="rg", name="rg")
                    nc.vector.memset(rg, 1.0)
                    for qc in range(4):
                        nc.vector.tensor_copy(out=oTu[n][r0:r0 + 64,
                                                         qc * 512:(qc + 1) * 512],
                                              in_=pvt[qc][0:64, :])
                        nc.vector.tensor_copy(out=rg[qc * 32:qc * 32 + 1, :],
                                              in_=pvt[qc][64:65, :])
                    rinv = nrmpool.tile([128, 512], f32, tag="ri", name="rinv")
                    nc.vector.reciprocal(out=rinv, in_=rg)
                    drv = drmpool.tile([4, 512], f32, tag="drv", name="drv")
                    dma.dma_start(out=drv,
                                  in_=rinv.rearrange("(a b) f -> a b f", b=32)[:, 0, :])
                    ib = nrmpool.tile([128, L], f32, tag="ib", bufs=2, name="ib")
                    dma.dma_start(out=ib[r0:r0 + 64, :],
                                  in_=drv.flatten()[:].partition_broadcast(64))
                    for qc in range(4):
                        sl = slice(qc * 512, (qc + 1) * 512)
                        nc.vector.tensor_mul(oT[n][r0:r0 + 64, sl],
                                             oTu[n][r0:r0 + 64, sl],
                                             ib[r0:r0 + 64, sl])

            # ---- Phase out-projection ----
            with tc.tile_pool(name="psC", bufs=1, space="PSUM") as psC, \
                 tc.tile_pool(name="otp", bufs=4) as otpool:
                for mt_i in range(8):
                    ot = otpool.tile([128, L], f32, tag="ot", bufs=2, name="ot")
                    for qc in range(4):
                        op = psC.tile([128, 512], f32, tag="op", bufs=4, name="op")
                        for n2 in range(2):
                            nc.tensor.matmul(
                                op,
                                lhsT=woS[n2][:, mt_i * 128:(mt_i + 1) * 128],
                                rhs=oT[n2][:, qc * 512:(qc + 1) * 512],
                                start=(n2 == 0), stop=(n2 == 1))
                        nc.vector.tensor_copy(out=ot[:, qc * 512:(qc + 1) * 512],
                                              in_=op)
                    dma.dma_start(out=outT[mt_i * 128:(mt_i + 1) * 128, :], in_=ot)

    nc.finalize()
    return nc


def prep_inputs(query, key, value, attn_mask, action_ids, time_deltas,
                Wq, bq, Wk, bk, Wv, bv, Wu, bu, Wo, bo,
                action_emb, Wap, bap, td_emb, td_gate):
    """Host-side sharding: build the 8 per-core input maps."""
    query = np.asarray(query, np.float32)
    key = np.asarray(key, np.float32)
    value = np.asarray(value, np.float32)
    attn_mask = np.asarray(attn_mask)
    action_ids = np.asarray(action_ids)
    time_deltas = np.asarray(time_deltas)

    sig_gate = 1.0 / (1.0 + np.exp(-np.float64(td_gate)))
    with_mask = not bool(attn_mask.all())

    xq_b, xk_b, xv_b, cb_b, mk_b = [], [], [], [], []
    for b in range(B):
        ae = np.asarray(action_emb, np.float32)[action_ids[b]]      # [L, 16]
        xqa = np.zeros((DPAD, L), BF16)
        xqa[:D] = query[b].T.astype(BF16)
        xqa[D:D + 16] = ae.T.astype(BF16)
        xqa[D + 16] = BF16(1.0)
        xq_b.append(xqa)
        xka = np.zeros((DPAD, L), BF16)
        xka[:D] = key[b].T.astype(BF16)
        xka[D] = BF16(1.0)
        xk_b.append(xka)
        xva = np.zeros((DPAD, L), BF16)
        xva[:D] = value[b].T.astype(BF16)
        xva[D] = BF16(1.0)
        xv_b.append(xva)
        tdc = np.clip(time_deltas[b].astype(np.int64), 0, td_emb.shape[0] - 1)
        cb_b.append((sig_gate * np.asarray(td_emb, np.float32)[tdc]).astype(np.float32))
        if with_mask:
            m = np.where(attn_mask[b], np.float32(0.0), np.float32(-1e9))
            mk_b.append(np.ascontiguousarray(m.T))                  # [k, q]

    wq_a = np.zeros((DPAD, D), np.float32)
    wq_a[:D] = Wq
    wq_a[D + 16] = bq
    wu_a = np.zeros((DPAD, D), np.float32)
    wu_a[:D] = Wu
    wu_a[D:D + 16] = Wap
    wu_a[D + 16] = np.asarray(bu) + np.asarray(bap)
    wk_a = np.zeros((DPAD, D), np.float32)
    wk_a[:D] = Wk
    wk_a[D] = bk
    wv_a = np.zeros((DPAD, D), np.float32)
    wv_a[:D] = Wv
    wv_a[D] = bv

    # RoPE tables in [dh, pos] orientation, duplicated for the 2-head packing.
    # sin table carries the rotate_half sign: rows d<32 of each 64-block hold
    # -sin (they multiply q[d+32]), rows d>=32 hold +sin (multiply q[d-32]).
    inv_freq = 1.0 / (10000.0 ** (np.arange(0, DH, 2, dtype=np.float64) / DH))
    pos = np.arange(L, dtype=np.float64)
    freqs = pos[None, :] * inv_freq[:, None]            # [32, L]
    cos_t = np.repeat(np.cos(freqs), 2, axis=0)[:DH]    # [64, L]
    sin_t = np.repeat(np.sin(freqs), 2, axis=0)[:DH]
    ss_t = sin_t.copy()
    ss_t[0:32] = -ss_t[0:32]
    cs_t = np.ascontiguousarray(np.concatenate([cos_t, cos_t], 0), np.float32)
    sn_t = np.ascontiguousarray(np.concatenate([ss_t, ss_t], 0), np.float32)

    in_maps = []
    for c in range(NCORES):
        b, hg = c // 4, c % 4
        csl = slice(hg * NG, (hg + 1) * NG)
        cbc = cb_b[b][:, hg * 4:(hg + 1) * 4]                       # [L, 4]
        cbc = cbc.reshape(16, 128, 4).transpose(1, 0, 2).reshape(128, 64)
        m = {
            "xq": xq_b[b], "xk": xk_b[b], "xv": xv_b[b],
            "wq": wq_a[:, csl].astype(BF16), "wu": wu_a[:, csl].astype(BF16),
            "wk": wk_a[:, csl].astype(BF16), "wv": wv_a[:, csl].astype(BF16),
            "wo": np.asarray(Wo, np.float32)[csl, :].astype(BF16),
            "cb": np.ascontiguousarray(cbc, np.float32),
            "cs": cs_t, "sn": sn_t,
        }
        if with_mask:
            m["mk"] = mk_b[b]
        in_maps.append(m)
    return in_maps, with_mask


def gather_output(results, bo):
    """Sum head-group partials per batch, transpose, add bo."""
    out = np.empty((B, L, D), np.float32)
    for b in range(B):
        acc = results[b * 4]["outT"].astype(np.float32).copy()
        for g in range(1, 4):
            acc += results[b * 4 + g]["outT"]
        out[b] = acc.T + np.asarray(bo, np.float32)
    return out


def kernel(**inputs):
    from concourse.bass_utils import run_bass_kernel_spmd

    in_maps, with_mask = prep_inputs(**inputs)
    nc = build_bass(with_mask)
    res = run_bass_kernel_spmd(nc, in_maps, core_ids=list(range(NCORES)))
    return gather_output(res.results, inputs["bo"])


# revision 30
# speedup vs baseline: 1.4255x; 1.2002x over previous
"""Trainium2 Bass kernel for FlashMultiHeadAttention (B=2, L=2048, D=1024, H=16, Dh=64).

Sharding: 8 cores = 2 (batch) x 4 (head groups of 4 heads).
Per core (batch b, head group hg, 4 heads):
  - Q^T, K^T projections ([256, L], head dim on partitions) with RoPE applied
    during PSUM evacuation via partition-shifted DVE multiplies against
    cos / signed-sin tables; U, V projected in natural [L, 256] layout with
    biases and the action gate folded in via augmented contraction rows.
  - Scores computed transposed (S^T[k, q]) so the learned time-delta bias is a
    per-partition ACT bias fused into the exp instruction (scale+bias+exp+cast
    in one op). P^T feeds PV directly - no PE transposes anywhere.
  - PV carries an extra all-ones column producing softmax denominators;
    normalization uses a batched reciprocal + DRAM-broadcast of 1/r.
  - Row-sliced output projection -> partial outT [1024, 2048] fp32.
Host sums the 4 head-group partials per batch and adds bo.

A single PSUM pool (4x 1-bank "sm" slots + 2x 2-bank "st" slots) is shared by
every phase so there are no pool-release barriers; phases overlap by dataflow
and the PE never idles long enough to drop the HAM clock.
"""

import sys

if "/opt/trn_rl_repo" not in sys.path:
    sys.path.insert(0, "/opt/trn_rl_repo")

import numpy as np
import ml_dtypes

BF16 = ml_dtypes.bfloat16

B = 2
L = 2048
D = 1024
H = 16
DH = 64
NG = 256          # head dims per group (4 heads)
DPAD = 1152       # padded contraction rows (9 * 128)
NCORES = 8
SCALE = DH ** -0.5


def build_bass(with_mask: bool):
    """Build the single-core SPMD Bass program (same program on all 8 cores)."""
    import concourse.mybir as mybir
    from concourse import bacc
    from concourse.tile import TileContext

    f32 = mybir.dt.float32
    bf16 = mybir.dt.bfloat16
    EXP = mybir.ActivationFunctionType.Exp
    TANH = mybir.ActivationFunctionType.Tanh

    nc = bacc.Bacc(None, target_bir_lowering=False)

    xq = nc.dram_tensor("xq", [DPAD, L], bf16, kind="ExternalInput")
    xk = nc.dram_tensor("xk", [DPAD, L], bf16, kind="ExternalInput")
    xv = nc.dram_tensor("xv", [DPAD, L], bf16, kind="ExternalInput")
    wq = nc.dram_tensor("wq", [DPAD, NG], bf16, kind="ExternalInput")
    wu = nc.dram_tensor("wu", [DPAD, NG], bf16, kind="ExternalInput")
    wk = nc.dram_tensor("wk", [DPAD, NG], bf16, kind="ExternalInput")
    wv = nc.dram_tensor("wv", [DPAD, NG], bf16, kind="ExternalInput")
    wo = nc.dram_tensor("wo", [NG, D], bf16, kind="ExternalInput")
    cb = nc.dram_tensor("cb", [128, 64], f32, kind="ExternalInput")
    cs = nc.dram_tensor("cs", [128, L], f32, kind="ExternalInput")
    sn = nc.dram_tensor("sn", [128, L], f32, kind="ExternalInput")
    mk = None
    if with_mask:
        mk = nc.dram_tensor("mk", [L, L], f32, kind="ExternalInput")
    outT = nc.dram_tensor("outT", [D, L], f32, kind="ExternalOutput")

    dma = nc.default_dma_engine

    def rope_evac(pps, dest, csS, snS, s, pool):
        """dest[:, s] (bf16) = pps*cos + rotate_half(pps)*signed_sin."""
        tc_ = pool.tile([128, 512], f32, tag="tc", name="tc_")
        tr_ = pool.tile([128, 512], f32, tag="tr", name="tr_")
        nc.vector.tensor_mul(tc_, pps, csS[:, s])
        for blk in (0, 64):
            nc.vector.tensor_mul(tr_[blk:blk + 32],
                                 pps[blk + 32:blk + 64, :], snS[blk:blk + 32, s])
            nc.vector.tensor_mul(tr_[blk + 32:blk + 64],
                                 pps[blk:blk + 32, :], snS[blk + 32:blk + 64, s])
        nc.vector.tensor_add(dest[:, s], tc_, tr_)

    with TileContext(nc) as tc:
        with tc.tile_pool(name="persist", bufs=1) as persist, \
             tc.tile_pool(name="xkp", bufs=1) as xkpool, \
             tc.tile_pool(name="ps", bufs=1, space="PSUM") as ps, \
             tc.tile_pool(name="ev", bufs=2) as ev, \
             tc.tile_pool(name="ptp", bufs=6) as ptpool, \
             tc.tile_pool(name="nrm", bufs=2) as nrmpool, \
             tc.tile_pool(name="drm", bufs=2, space="DRAM") as drmpool, \
             tc.tile_pool(name="mkp", bufs=4) as mkpool:
            qT = [persist.tile([128, L], bf16, name=f"qT{n}") for n in range(2)]
            kT = [persist.tile([128, L], bf16, name=f"kT{n}") for n in range(2)]
            vg = persist.tile([128, 16 * 260], bf16, name="vg")
            oT = [persist.tile([128, L], bf16, name=f"oT{n}") for n in range(2)]
            sig = [persist.tile([128, 1024], bf16, name=f"sig{qc}") for qc in range(4)]
            cbS = persist.tile([128, 64], f32, name="cbS")
            csS = persist.tile([128, L], f32, name="csS")
            snS = persist.tile([128, L], f32, name="snS")
            woS = [persist.tile([128, D], bf16, name=f"woS{n2}") for n2 in range(2)]
            wqS = persist.tile([128, 9 * NG], bf16, name="wqS")
            wuS = persist.tile([128, 9 * NG], bf16, name="wuS")
            wkS = persist.tile([128, 9 * NG], bf16, name="wkS")
            wvS = persist.tile([128, 9 * NG], bf16, name="wvS")
            dma.dma_start(out=wqS.rearrange("p (c n) -> p c n", n=NG),
                          in_=wq.rearrange("(c p) n -> p c n", p=128))
            wqS3 = wqS.rearrange("p (c n) -> p c n", n=NG)
            wuS3 = wuS.rearrange("p (c n) -> p c n", n=NG)
            wkS3 = wkS.rearrange("p (c n) -> p c n", n=NG)
            wvS3 = wvS.rearrange("p (c n) -> p c n", n=NG)

            vg4 = vg.rearrange("p (t h e) -> p t h e", h=4, e=65)
            nc.vector.memset(vg4[:, :, :, 64:65], 1.0)

            xkS = xkpool.tile([128, 9 * L], bf16, name="xkS")
            xkS3 = xkS.rearrange("p (c q) -> p c q", q=L)

            def sm_tile(name):
                return ps.tile([128, 512], f32, tag="sm", bufs=4, name=name)

            # ---- QU: Q^T (+RoPE) then U + sigmoid, per q-chunk ----
            with tc.tile_pool(name="xqp", bufs=1) as xqpool:
                xqS = xqpool.tile([128, 9 * L], bf16, name="xqS")
                xqS3 = xqS.rearrange("p (c q) -> p c q", q=L)
                for qc in range(4):
                    s = slice(qc * 512, (qc + 1) * 512)
                    for d in range(9):
                        dma.dma_start(out=xqS3[:, d, s],
                                      in_=xq[d * 128:(d + 1) * 128, s])
                dma.dma_start(out=wuS.rearrange("p (c n) -> p c n", n=NG),
                              in_=wu.rearrange("(c p) n -> p c n", p=128))
                # cos/sin/bias/wo on the second HWDGE ring (ACT engine) so
                # they arrive in parallel with the xq stream
                nc.scalar.dma_start(out=csS, in_=cs[:, :])
                nc.scalar.dma_start(out=snS, in_=sn[:, :])
                nc.scalar.dma_start(out=cbS, in_=cb[:, :])
                for n2 in range(2):
                    nc.scalar.dma_start(out=woS[n2], in_=wo[n2 * 128:(n2 + 1) * 128, :])
                for wt_sb, wt_dr in ((wkS, wk), (wvS, wv)):
                    dma.dma_start(out=wt_sb.rearrange("p (c n) -> p c n", n=NG),
                                  in_=wt_dr.rearrange("(c p) n -> p c n", p=128))
                for qc in range(4):
                    s = slice(qc * 512, (qc + 1) * 512)
                    for d in range(9):
                        dma.dma_start(out=xkS3[:, d, s],
                                      in_=xk[d * 128:(d + 1) * 128, s])

                for qc in range(4):
                    s = slice(qc * 512, (qc + 1) * 512)
                    qps = [sm_tile(f"qps{n}") for n in range(2)]
                    for d in range(9):
                        xt = xqS3[:, d, s]
                        for n in range(2):
                            nc.tensor.matmul(qps[n],
                                             lhsT=wqS3[:, d, n * 128:(n + 1) * 128],
                                             rhs=xt, start=(d == 0), stop=(d == 8))
                    for n in range(2):
                        rope_evac(qps[n], qT[n], csS, snS, s, ev)
                    ups = [sm_tile(f"ups{i}") for i in range(4)]
                    for d in range(9):
                        xt = xqS3[:, d, s]
                        for i in range(4):
                            nc.tensor.matmul(ups[i][:, 0:256],
                                             lhsT=xt[:, i * 128:(i + 1) * 128],
                                             rhs=wuS3[:, d, :],
                                             start=(d == 0), stop=(d == 8))
                    # sigmoid(u) = 0.5*tanh(0.5*u) + 0.5
                    eu = ev.tile([128, 1024], f32, tag="eu", name="eu")
                    for i in range(4):
                        nc.scalar.activation(out=eu[:, i * 256:(i + 1) * 256],
                                             in_=ups[i][:, 0:256], func=TANH,
                                             scale=0.5)
                    nc.vector.tensor_scalar(sig[qc], eu, 0.5, 0.5,
                                            mybir.AluOpType.mult,
                                            mybir.AluOpType.add)

            # keep the PE clock warm across the QU tail (tanh/sig + rope DVE)
            wt0 = ps.tile([128, 512], f32, tag="sm", bufs=4, name="warm0")
            for j0 in range(24):
                nc.tensor.matmul(wt0, lhsT=woS[0][:, 0:128], rhs=woS[0][:, 0:512],
                                 start=(j0 == 0), stop=(j0 == 23))

            # ---- KV: K^T (+RoPE) then V + gating, per q-chunk ----
            with tc.tile_pool(name="xvp", bufs=1) as xvpool:
                xvS = xvpool.tile([128, 9 * L], bf16, name="xvS")
                xvS3 = xvS.rearrange("p (c q) -> p c q", q=L)
                for qc in range(4):
                    s = slice(qc * 512, (qc + 1) * 512)
                    for d in range(9):
                        dma.dma_start(out=xvS3[:, d, s],
                                      in_=xv[d * 128:(d + 1) * 128, s])
                for qc in range(4):
                    s = slice(qc * 512, (qc + 1) * 512)
                    kps = [sm_tile(f"kps{n}") for n in range(2)]
                    for d in range(9):
                        xtk = xkS3[:, d, s]
                        for n in range(2):
                            nc.tensor.matmul(kps[n],
                                             lhsT=wkS3[:, d, n * 128:(n + 1) * 128],
                                             rhs=xtk, start=(d == 0), stop=(d == 8))
                    for n in range(2):
                        rope_evac(kps[n], kT[n], csS, snS, s, ev)
                    vps = [sm_tile(f"vps{i}") for i in range(4)]
                    for d in range(9):
                        xtv = xvS3[:, d, s]
                        for i in range(4):
                            nc.tensor.matmul(vps[i][:, 0:256],
                                             lhsT=xtv[:, i * 128:(i + 1) * 128],
                                             rhs=wvS3[:, d, :],
                                             start=(d == 0), stop=(d == 8))
                    for i in range(4):
                        kt_g = qc * 4 + i
                        vsrc = vps[i][:, 0:256].rearrange("p (h e) -> p h e", e=64)
                        ssrc = sig[qc][:, i * 256:(i + 1) * 256].rearrange(
                            "p (h e) -> p h e", e=64)
                        nc.vector.tensor_mul(vg4[:, kt_g, :, 0:64], vsrc, ssrc)

            # ---- Attention ----
            for h in range(4):
                n = h // 2
                r0 = (h % 2) * 64
                pvt = [ps.tile([65, 512], f32, tag="sm", bufs=4, name=f"pvt{qc}")
                       for qc in range(4)]
                for kt in range(16):
                    for hq in range(2):
                        st = ps.tile([128, 1024], f32, tag="st", bufs=2, name="st")
                        for s2 in range(2):
                            q0 = hq * 1024 + s2 * 512
                            nc.tensor.matmul(
                                st[:, s2 * 512:(s2 + 1) * 512],
                                lhsT=kT[n][r0:r0 + 64, kt * 128:(kt + 1) * 128],
                                rhs=qT[n][r0:r0 + 64, q0:q0 + 512],
                                start=True, stop=True)
                        if with_mask:
                            mt = mkpool.tile([128, 1024], f32, tag="mt", name="mt")
                            dma.dma_start(
                                out=mt,
                                in_=mk[kt * 128:(kt + 1) * 128,
                                       hq * 1024:(hq + 1) * 1024])
                            nc.vector.tensor_add(st, st, mt)
                        pt = ptpool.tile([128, 1024], bf16, tag="pt", name="pt")
                        nc.scalar.activation(out=pt, in_=st, func=EXP,
                                             scale=SCALE,
                                             bias=cbS[:, kt * 4 + h:kt * 4 + h + 1])
                        for s2 in range(2):
                            qc = hq * 2 + s2
                            nc.tensor.matmul(
                                pvt[qc],
                                lhsT=vg[:, kt * 260 + h * 65:kt * 260 + h * 65 + 65],
                                rhs=pt[:, s2 * 512:(s2 + 1) * 512],
                                start=(kt == 0), stop=(kt == 15))
                # evacuate numerators + denominators; batched reciprocal.
                rg = nrmpool.tile([128, 512], f32, tag="rg", name="rg")
                nc.gpsimd.memset(rg, 1.0)
                for qc in range(4):
                    nc.vector.tensor_copy(out=oT[n][r0:r0 + 64,
                                                    qc * 512:(qc + 1) * 512],
                                          in_=pvt[qc][0:64, :])
                    nc.vector.tensor_copy(out=rg[qc * 32:qc * 32 + 1, :],
                                          in_=pvt[qc][64:65, :])
                rinv = nrmpool.tile([128, 512], f32, tag="ri", name="rinv")
                nc.vector.reciprocal(out=rinv, in_=rg)
                drv = drmpool.tile([4, 512], f32, tag="drv", name="drv")
                dma.dma_start(out=drv,
                              in_=rinv.rearrange("(a b) f -> a b f", b=32)[:, 0, :])
                ib = nrmpool.tile([128, L], f32, tag="ib", bufs=2, name="ib")
                dma.dma_start(out=ib[r0:r0 + 64, :],
                              in_=drv.flatten()[:].partition_broadcast(64))
                for qc in range(4):
                    sl = slice(qc * 512, (qc + 1) * 512)
                    nc.vector.tensor_mul(oT[n][r0:r0 + 64, sl],
                                         oT[n][r0:r0 + 64, sl],
                                         ib[r0:r0 + 64, sl])

            # keep the PE clock warm across the normalize tail of head 3
            wt_ = ps.tile([128, 512], f32, tag="sm", bufs=4, name="warm")
            for j_ in range(44):
                nc.tensor.matmul(wt_, lhsT=woS[0][:, 0:128], rhs=woS[0][:, 0:512],
                                 start=(j_ == 0), stop=(j_ == 43))

            # ---- Out-projection ----
            with tc.tile_pool(name="otp", bufs=2) as otpool:
                for mt_i in range(8):
                    ot = otpool.tile([128, L], f32, tag="ot", name="ot")
                    ops = [sm_tile(f"op{qc}") for qc in range(4)]
                    for n2 in range(2):
                        for qc in range(4):
                            nc.tensor.matmul(
                                ops[qc],
                                lhsT=woS[n2][:, mt_i * 128:(mt_i + 1) * 128],
                                rhs=oT[n2][:, qc * 512:(qc + 1) * 512],
                                start=(n2 == 0), stop=(n2 == 1))
                    for qc in range(4):
                        nc.vector.tensor_copy(out=ot[:, qc * 512:(qc + 1) * 512],
                                              in_=ops[qc])
                    dma.dma_start(out=outT[mt_i * 128:(mt_i + 1) * 128, :], in_=ot)

    nc.finalize()
    return nc


def prep_inputs(query, key, value, attn_mask, action_ids, time_deltas,
                Wq, bq, Wk, bk, Wv, bv, Wu, bu, Wo, bo,
                action_emb, Wap, bap, td_emb, td_gate):
    """Host-side sharding: build the 8 per-core input maps."""
    query = np.asarray(query, np.float32)
    key = np.asarray(key, np.float32)
    value = np.asarray(value, np.float32)
    attn_mask = np.asarray(attn_mask)
    action_ids = np.asarray(action_ids)
    time_deltas = np.asarray(time_deltas)

    sig_gate = 1.0 / (1.0 + np.exp(-np.float64(td_gate)))
    with_mask = not bool(attn_mask.all())

    xq_b, xk_b, xv_b, cb_b, mk_b = [], [], [], [], []
    for b in range(B):
        ae = np.asarray(action_emb, np.float32)[action_ids[b]]      # [L, 16]
        xqa = np.zeros((DPAD, L), BF16)
        xqa[:D] = query[b].T.astype(BF16)
        xqa[D:D + 16] = ae.T.astype(BF16)
        xqa[D + 16] = BF16(1.0)
        xq_b.append(xqa)
        xka = np.zeros((DPAD, L), BF16)
        xka[:D] = key[b].T.astype(BF16)
        xka[D] = BF16(1.0)
        xk_b.append(xka)
        xva = np.zeros((DPAD, L), BF16)
        xva[:D] = value[b].T.astype(BF16)
        xva[D] = BF16(1.0)
        xv_b.append(xva)
        tdc = np.clip(time_deltas[b].astype(np.int64), 0, td_emb.shape[0] - 1)
        cb_b.append((sig_gate * np.asarray(td_emb, np.float32)[tdc]).astype(np.float32))
        if with_mask:
            m = np.where(attn_mask[b], np.float32(0.0), np.float32(-1e9))
            mk_b.append(np.ascontiguousarray(m.T))                  # [k, q]

    wq_a = np.zeros((DPAD, D), np.float32)
    wq_a[:D] = Wq
    wq_a[D + 16] = bq
    wu_a = np.zeros((DPAD, D), np.float32)
    wu_a[:D] = Wu
    wu_a[D:D + 16] = Wap
    wu_a[D + 16] = np.asarray(bu) + np.asarray(bap)
    wk_a = np.zeros((DPAD, D), np.float32)
    wk_a[:D] = Wk
    wk_a[D] = bk
    wv_a = np.zeros((DPAD, D), np.float32)
    wv_a[:D] = Wv
    wv_a[D] = bv

    # RoPE tables in [dh, pos] orientation, duplicated for the 2-head packing.
    # sin table carries the rotate_half sign: rows d<32 of each 64-block hold
    # -sin (they multiply q[d+32]), rows d>=32 hold +sin (multiply q[d-32]).
    inv_freq = 1.0 / (10000.0 ** (np.arange(0, DH, 2, dtype=np.float64) / DH))
    pos = np.arange(L, dtype=np.float64)
    freqs = pos[None, :] * inv_freq[:, None]            # [32, L]
    cos_t = np.repeat(np.cos(freqs), 2, axis=0)[:DH]    # [64, L]
    sin_t = np.repeat(np.sin(freqs), 2, axis=0)[:DH]
    ss_t = sin_t.copy()
    ss_t[0:32] = -ss_t[0:32]
    cs_t = np.ascontiguousarray(np.concatenate([cos_t, cos_t], 0), np.float32)
    sn_t = np.ascontiguousarray(np.concatenate([ss_t, ss_t], 0), np.float32)

    in_maps = []
    for c in range(NCORES):
        b, hg = c // 4, c % 4
        csl = slice(hg * NG, (hg + 1) * NG)
        cbc = cb_b[b][:, hg * 4:(hg + 1) * 4]                       # [L, 4]
        cbc = cbc.reshape(16, 128, 4).transpose(1, 0, 2).reshape(128, 64)
        m = {
            "xq": xq_b[b], "xk": xk_b[b], "xv": xv_b[b],
            "wq": wq_a[:, csl].astype(BF16), "wu": wu_a[:, csl].astype(BF16),
            "wk": wk_a[:, csl].astype(BF16), "wv": wv_a[:, csl].astype(BF16),
            "wo": np.asarray(Wo, np.float32)[csl, :].astype(BF16),
            "cb": np.ascontiguousarray(cbc, np.float32),
            "cs": cs_t, "sn": sn_t,
        }
        if with_mask:
            m["mk"] = mk_b[b]
        in_maps.append(m)
    return in_maps, with_mask


def gather_output(results, bo):
    """Sum head-group partials per batch, transpose, add bo."""
    out = np.empty((B, L, D), np.float32)
    for b in range(B):
        acc = results[b * 4]["outT"].astype(np.float32).copy()
        for g in range(1, 4):
            acc += results[b * 4 + g]["outT"]
        out[b] = acc.T + np.asarray(bo, np.float32)
    return out


def kernel(**inputs):
    from concourse.bass_utils import run_bass_kernel_spmd

    in_maps, with_mask = prep_inputs(**inputs)
    nc = build_bass(with_mask)
    res = run_bass_kernel_spmd(nc, in_maps, core_ids=list(range(NCORES)))
    return gather_output(res.results, inputs["bo"])
